# revision 75
# baseline (speedup 1.0000x reference)
"""ChebNetII GNN kernel for 8 Trainium2 NeuronCores (Bass/Tile).

Computation (matches the reference):
    h   = relu(X @ W1 + b1) @ W2 + b2                  # MLP, [N, H]
    coe = 2/(K+1) * T @ relu(temp)                     # Chebyshev coefficients
    P x = segment_sum(norm_e * x[src], dst)            # normalized propagation
        norm_e = -dis[src] * dis[dst],  dis = deg_src^{-1/2}
    Tx_0 = h; Tx_1 = P h; Tx_i = 2 P Tx_{i-1} - Tx_{i-2}
    out = coe0/2 * Tx_0 + sum_i coe_i * Tx_i

Fast path: when relu(temp) is constant (the spec pins temp = ones),
discrete Chebyshev orthogonality makes coe[1:] vanish identically, so
out == (coe0/2) * h and no propagation is needed. The kernel detects
this from temp at runtime (coe computed in float64; propagation terms
are bounded by ||Tx_i|| <= ||h||, so |coe[1:]| < 1e-5|coe0| bounds the
skipped contribution at ~1e-4 relative, far inside the 2e-2 gate) and
runs an MLP-only SPMD program: nodes row-sharded over 8 cores, X cast
host-side to bf16, stage-1/2 matmuls in bf16 with a 65th ones-row
produced by the activation bias trick (bias[64]=1, zero W1 column), and
outputs streamed back per tile as bf16 with host upcast to f32. The
weights ride in the first columns of input piece 0 so a single SP/HWDGE
DMA chain feeds the whole pipeline; output flushes alternate between
the Pool (SWDGE) and SP (HWDGE) issue queues; stage-2 is emitted two
tiles behind stage-1 so PE's FIFO never head-of-line blocks on an
unfinished activation.

Fallback (any other temp) - the full propagation kernel: per step, each
core's shard of y = dis * Tx is AllGathered into a DRAM table; each
core then gathers its in-edge source rows with batched dma_gather
(256B rows; the int16 index range forces a low/high table split at
32768), aggregates them per 64-destination window with one-hot matmuls
on the TensorEngine (one-hots built on DVE via is_equal against an
iota constant, two windows packed per PSUM tile via column tiling),
and applies the recurrence with fused DVE post-ops. Because norm
factorizes as dis[src]*dis[dst], the table is pre-scaled by dis so
aggregation is an unweighted segment-sum followed by a per-node
post-scale.
"""

import numpy as np

# Problem constants (hardcoded per the task contract).
N = 50000
E = 800000
F_IN = 256
H = 64
K = 6
N_CORES = 8

WIN = 64              # destination-window width (= one-hot / matmul M)
SPLIT = 32768         # low/high table split (int16 dma_gather indices)
PAIRS_PER_GATHER = 5  # window-pairs batched into one dma_gather pair
USE_BF16 = True       # bf16 table/S/gather payload (PSUM accumulation stays f32)


def _cheb_coe(temp):
    j = np.arange(K + 1)
    xs = np.cos((K - j + 0.5) * np.pi / (K + 1))
    T = np.zeros((K + 1, K + 1), dtype=np.float64)
    T[0] = 1.0
    T[1] = xs
    for i in range(2, K + 1):
        T[i] = 2.0 * xs * T[i - 1] - T[i - 2]
    return ((2.0 / (K + 1)) * (T @ np.maximum(temp.astype(np.float64), 0.0))).astype(
        np.float32
    )


def _wrap_idx(flat):
    """[128*C] int16 slot array -> [128, 8*C] wrapped layout: index i lives at
    (i%16, i//16), replicated across the 8 groups of 16 partitions."""
    w = flat.reshape(-1, 16).T  # [16, len/16]
    return np.tile(w, (8, 1)).astype(np.int16)




def _balance_assignment(src, dst, n, n_cores, n_loc, nw, split):
    """Two-phase balanced assignment of nodes to (core, window-position).

    Phase A snakes nodes over cores by total in-degree; with the low/high
    table split pinned to a core boundary, this fixes every edge's L/H
    class. Phase B packs each core's windows so L and H edge counts land
    just under multiples of 128 (minimizes gather padding). Returns
    perm[node] = global position.
    """
    indeg = np.bincount(dst, minlength=n).astype(np.int64)
    order = np.argsort(-indeg)
    core_of = np.empty(n, dtype=np.int64)
    snake = np.concatenate([np.arange(n_cores), np.arange(n_cores)[::-1]])
    core_of[order] = snake[np.arange(n) % (2 * n_cores)]
    # fix per-core counts to exactly n_loc (snake may be off when n % (2*n_cores))
    for c in range(n_cores):
        pass
    counts = np.bincount(core_of, minlength=n_cores)
    # rebalance counts exactly: move lightest nodes from over-full cores
    for c in range(n_cores):
        while counts[c] > n_loc:
            # move the lowest-degree node of core c to an under-full core
            cand = np.where(core_of == c)[0]
            v = cand[np.argmin(indeg[cand])]
            tgt = int(np.argmin(counts))
            core_of[v] = tgt
            counts[c] -= 1
            counts[tgt] += 1

    split_core = split // n_loc
    hi_edge = (core_of[src] >= split_core).astype(np.int64)
    inl = np.bincount(dst[hi_edge == 0], minlength=n).astype(np.int64)
    inh = np.bincount(dst[hi_edge == 1], minlength=n).astype(np.int64)

    perm = np.empty(n, dtype=np.int64)
    for c in range(n_cores):
        nodes = np.where(core_of == c)[0]
        L = inl[nodes]
        Hh = inh[nodes]
        lmax = int(L.max()) + 1 if len(L) else 1
        hmax = int(Hh.max()) + 1 if len(Hh) else 1
        buckets = np.zeros((lmax, hmax), dtype=np.int64)
        np.add.at(buckets, (L, Hh), 1)
        bucket_nodes = {}
        for i, (lv, hv) in enumerate(zip(L, Hh)):
            bucket_nodes.setdefault((int(lv), int(hv)), []).append(i)

        caps = np.full(nw, WIN, dtype=np.int64)
        caps[-1] = n_loc - WIN * (nw - 1)
        remL, remH = int(L.sum()), int(Hh.sum())
        rem_bins = nw
        out_bins = []
        for b in range(nw):
            cap = int(caps[b])
            frac = cap / float(WIN)
            tl = 128.0 * np.floor(remL / rem_bins * frac / 128.0 + 0.5)
            th = 128.0 * np.floor(remH / rem_bins * frac / 128.0 + 0.5)
            if b == nw - 1:
                tl, th = remL, remH
            sl = sh = 0
            chosen = []
            for k in range(cap):
                slots = cap - k
                idl = (tl - sl) / slots
                idh = (th - sh) / slots
                lv0 = int(min(max(round(idl), 0), lmax - 1))
                best = None
                for dl in range(-8, 9):
                    lv = lv0 + dl
                    if lv < 0 or lv >= lmax:
                        continue
                    row = buckets[lv]
                    if not row.any():
                        continue
                    hv0 = int(min(max(round(idh), 0), hmax - 1))
                    for dh in range(-8, 9):
                        hv = hv0 + dh
                        if hv < 0 or hv >= hmax or row[hv] == 0:
                            continue
                        cost = abs(lv - idl) + abs(hv - idh)
                        if best is None or cost < best[0]:
                            best = (cost, lv, hv)
                if best is None:
                    nz = np.argwhere(buckets > 0)
                    d = np.abs(nz[:, 0] - idl) + np.abs(nz[:, 1] - idh)
                    lv, hv = (int(v) for v in nz[int(np.argmin(d))])
                else:
                    _, lv, hv = best
                buckets[lv, hv] -= 1
                i = bucket_nodes[(lv, hv)].pop()
                chosen.append(i)
                sl += lv
                sh += hv
            remL -= sl
            remH -= sh
            rem_bins -= 1
            out_bins.append((chosen, sl, sh))
        # repair pass: swap nodes between bins to pull H sums just under
        # multiples of 128 while keeping L sums balanced
        sl_arr = np.array([b_[1] for b_ in out_bins])
        sh_arr = np.array([b_[2] for b_ in out_bins])
        binof = np.empty(len(nodes), dtype=np.int64)
        for b, (ch, _, _) in enumerate(out_bins):
            for i in ch:
                binof[i] = b
        for _ in range(3):
            over = np.where((sh_arr % 128) > 0)[0]
            # bins sorted by how little they exceed a multiple (cheap to fix)
            over = sorted(over, key=lambda b: (sh_arr[b] % 128))
            changed = False
            for b in over:
                exc = int(sh_arr[b] % 128)
                if exc > 24:
                    continue
                head = np.where((128 - (sh_arr % 128)) % 128 >= exc)[0]
                head = [u for u in head if u != b]
                if not head:
                    continue
                # find i in b with hv ~ exc and j in u with hv ~ 0, |L diff| small
                cand_i = [i for i in out_bins[b][0]
                          if 0 < Hh[i] <= exc and binof[i] == b]
                if not cand_i:
                    continue
                i = max(cand_i, key=lambda i2: Hh[i2])
                hvi = int(Hh[i])
                done = False
                for u in head:
                    if (128 - (sh_arr[u] % 128)) % 128 < hvi:
                        continue
                    cand_j = [j for j in out_bins[u][0]
                              if Hh[j] == 0 and abs(int(L[j]) - int(L[i])) <= 2]
                    if cand_j:
                        j = cand_j[0]
                        out_bins[b][0].remove(i)
                        out_bins[u][0].remove(j)
                        out_bins[b][0].append(j)
                        out_bins[u][0].append(i)
                        dl = int(L[i]) - int(L[j])
                        sl_arr[b] -= dl
                        sl_arr[u] += dl
                        sh_arr[b] -= hvi
                        sh_arr[u] += hvi
                        binof[i], binof[j] = u, b
                        changed = True
                        done = True
                        break
            if not changed:
                break
        out_bins = [(out_bins[b][0], int(sl_arr[b]), int(sh_arr[b]))
                    for b in range(nw)]
        fulls = [b for b in range(nw) if caps[b] == WIN]
        fulls.sort(key=lambda b: (-((out_bins[b][1] + 127) // 128),
                                  -((out_bins[b][2] + 127) // 128),
                                  -out_bins[b][1]))
        rank = fulls + [b for b in range(nw) if caps[b] < WIN]
        for w, b in enumerate(rank):
            base_pos = c * n_loc + w * WIN
            for k2, i in enumerate(out_bins[b][0]):
                perm[nodes[i]] = base_pos + k2
    return perm

def _prepare_mlp(X, W1, b1, W2, b2, coe0, n=N, n_cores=N_CORES):
    """Fast-path prep: output reduces to (coe0/2) * (relu(X@W1+b1)@W2+b2).

    Node layout per core: node n_local <-> (partition p, slot b) with
    p = n_local // SL, b = n_local % SL, so each partition's output rows
    are contiguous in DRAM (>=512B full-rate descriptors per flush). The
    MLP runs in bf16 (inputs cast host-side; tolerance is 2e-2).
    """
    import ml_dtypes

    bf16 = ml_dtypes.bfloat16
    n_loc = n // n_cores
    NP, SL = 125, 50                  # partitions x slots; NP*SL == n_loc
    assert NP * SL == n_loc

    scale = coe0 / 2.0
    w1p = np.zeros((2, 128, H + 1), dtype=np.float32)
    w1p[:, :, :H] = np.asarray(W1, dtype=np.float32).reshape(2, 128, H)
    w1r = w1p.transpose(1, 0, 2).astype(bf16)         # [128, 2, H+1]
    w2b2 = (
        np.concatenate(
            [np.asarray(W2, dtype=np.float32),
             np.asarray(b2, dtype=np.float32)[None, :]],
            axis=0,
        )
        * np.float32(scale)
    ).astype(bf16)
    b1c = np.concatenate(
        [np.asarray(b1, dtype=np.float32), np.ones(1, dtype=np.float32)]
    )

    # weights and bias ride in the first EX columns of the xt stream so
    # piece 0 of the input DMA delivers them without extra DGE slots.
    # layout: cols [0,65) w1 chunks; fc1 cols [66,68) b1 as raw f32 bits
    # (4B-aligned, bitcast back on device); fc0 cols [68,132) w2b2.
    EX = 132
    ex = np.zeros((128, 2, EX), dtype=bf16)
    ex[:, :, 0 : H + 1] = w1r
    ex[0 : H + 1, 0, 68 : 68 + H] = w2b2
    ex_u16 = ex.view(np.uint16)
    ex_u16[0 : H + 1, 1, 66:68] = (
        b1c.astype("<f4").view("<u2").reshape(H + 1, 2)
    )

    Xb = np.asarray(X, dtype=np.float32).astype(bf16)
    in_maps = []
    for c in range(n_cores):
        xs = Xb[c * n_loc : (c + 1) * n_loc]          # [n_loc, F_IN]
        # xt[p_sb, fc, j] with column j = b*NP + p  <->  node p*SL + b
        xt = (
            xs.T.reshape(2, 128, NP, SL)
            .transpose(1, 0, 3, 2)                    # [128, 2, SL, NP]
            .reshape(128, 2, n_loc)
        )
        in_maps.append({"xt": np.concatenate([ex, xt], axis=2)})

    plan = {"mlp_only": True, "n": n, "n_loc": n_loc, "np_": NP, "sl": SL,
            "n_cores": n_cores}
    return plan, in_maps


def _build_mlp(plan):
    """MLP-only SPMD Bass program: out = scaled(relu(X@W1+b1)@W2+b2)."""
    from concourse import bacc, bass, mybir

    import concourse.tile as tile

    n_loc = plan["n_loc"]
    NP, SL = plan["np_"], plan["sl"]
    n_cores = plan["n_cores"]
    f32 = mybir.dt.float32
    bf16 = mybir.dt.bfloat16

    nc = bacc.Bacc(
        "TRN2", target_bir_lowering=False, debug=False, num_devices=n_cores
    )

    EX = 132
    xt_d = nc.dram_tensor("xt", [128, 2, EX + n_loc], bf16, kind="ExternalInput")
    out_d = nc.dram_tensor("out", [n_loc, H], bf16, kind="ExternalOutput")

    # compute tiles: 4 blocks of NP nodes (500); input pieces span 1-2 tiles
    # (HWDGE charges a fixed ~625ns per DMA, so fewer/bigger DMAs win);
    # output flushes go through the otherwise-idle Pool engine's SWDGE path.
    BPT = 4
    nb = n_loc // NP                       # 50 blocks
    tiles = [(0, 2)]                       # small first tile primes the pipe
    b0 = 2
    while b0 < nb:
        bt = min(BPT, nb - b0)
        tiles.append((b0, bt))
        b0 += bt
    pieces = [(0, 1)]                      # (first tile, #tiles) per input DMA
    t = 1
    while t < len(tiles):
        nt = min(2, len(tiles) - t)
        if len(tiles) - t <= 3:
            nt = 1
        pieces.append((t, nt))
        t += nt
    # flush after every tile, alternating SWDGE (Pool) and HWDGE (SP) issue
    # queues; the final flush covers only the small last tile so the tail
    # DMA is short
    flush_after = {}
    for i, t in enumerate(range(0, len(tiles))):
        flush_after[tiles[t][0]] = i % 2
    flush_after[tiles[-1][0]] = 1

    with tile.TileContext(nc) as tc:
        with tc.tile_pool(name="const", bufs=1) as cpool, \
             tc.tile_pool(name="pers", bufs=1) as pers, \
             tc.tile_pool(name="xin", bufs=5) as xpool, \
             tc.tile_pool(name="m1psum", bufs=3, space="PSUM") as mpsum, \
             tc.tile_pool(name="m2psum", bufs=4, space="PSUM") as apsum:

            from concourse import library_config
            nc.gpsimd.load_library(library_config.mlp)

            h1e = pers.tile([H + 1, n_loc], bf16)
            oacc = pers.tile([NP, nb, H], bf16)

            # tiny dummy activation: front-loads LoadActFuncSet at t~0 so
            # real activations aren't stalled behind a lazy 1.3us table load
            dum = cpool.tile([1, 1], f32)
            nc.vector.memset(dum[:], 0.0)
            nc.scalar.activation(
                out=dum[:], in_=dum[:],
                func=mybir.ActivationFunctionType.Relu,
                bias=dum[:, 0:1], scale=1.0,
            )

            # all input pieces stream on SP/HWDGE; piece 0 carries the
            # packed weights in its first EX columns, b1 follows as a tiny
            # f32 DMA so nothing queues behind the big input transfers
            xgs = {}
            first = True
            for (t0, nt) in pieces:
                c0 = tiles[t0][0] * NP
                ct = sum(tiles[t0 + k][1] for k in range(nt)) * NP
                if first:
                    xgp = cpool.tile([128, 2, EX + ct], bf16)
                    nc.sync.dma_start(
                        out=xgp[:], in_=xt_d[:, :, 0 : EX + ct]
                    )
                    xg0 = xgp
                    base = EX
                    first = False
                else:
                    xgp = xpool.tile([128, 2, ct], bf16, tag="xg")
                    nc.sync.dma_start(
                        out=xgp[:], in_=xt_d[:, :, EX + c0 : EX + c0 + ct]
                    )
                    base = 0
                for k in range(nt):
                    off = base + sum(tiles[t0 + j][1] for j in range(k)) * NP
                    xgs[t0 + k] = (xgp, off)
            w1t = xg0
            w2b2_ap = xg0[0 : H + 1, 0, 68 : 68 + H]
            b1t_ap = xg0[0 : H + 1, 1, 66:68].bitcast(f32)

            outr = out_d.rearrange("(p s) f -> p s f", p=NP)
            flst = {"fl0": 0}

            def stage2(t):
                """Emit stage-2 matmuls + copy + flush for tile t."""
                b0, bt = tiles[t]
                hp = apsum.tile([NP, bt, H], f32, tag="hp")
                for j in range(bt):
                    nc.tensor.matmul(
                        out=hp[:, j, :],
                        lhsT=h1e[:, (b0 + j) * NP : (b0 + j + 1) * NP],
                        rhs=w2b2_ap,
                        start=True,
                        stop=True,
                    )
                if t % 2 == 1:
                    nc.vector.tensor_copy(
                        out=oacc[:, b0 : b0 + bt, :], in_=hp[:]
                    )
                else:
                    nc.scalar.activation(
                        out=oacc[:, b0 : b0 + bt, :],
                        in_=hp[:],
                        func=mybir.ActivationFunctionType.Copy,
                    )
                if b0 in flush_after:
                    eng = nc.gpsimd if flush_after[b0] == 0 else nc.sync
                    eng.dma_start(
                        out=outr[:, flst["fl0"] : b0 + bt, :],
                        in_=oacc[:, flst["fl0"] : b0 + bt, :],
                    )
                    flst["fl0"] = b0 + bt

            # software-pipelined issue: stage-2 of tile t-2 is emitted between
            # later stage-1s so PE's FIFO queue never head-of-line blocks on
            # an activation that hasn't finished yet
            LAG = 2
            for t, (b0, bt) in enumerate(tiles):
                c0, ct = b0 * NP, bt * NP
                xgp, off = xgs[t]
                h1p = mpsum.tile([H + 1, ct], f32, tag="h1p")
                for fc in range(2):
                    nc.tensor.matmul(
                        out=h1p[:],
                        lhsT=xg0[:, fc, 0 : H + 1],
                        rhs=xgp[:, fc, off : off + ct],
                        start=(fc == 0),
                        stop=(fc == 1),
                    )
                # alternate relu between Act and DVE so neither engine's
                # serial throughput caps the per-tile cadence
                if t % 2 == 1:
                    nc.scalar.activation(
                        out=h1e[:, c0 : c0 + ct],
                        in_=h1p[:],
                        func=mybir.ActivationFunctionType.Relu,
                        bias=b1t_ap,
                        scale=1.0,
                    )
                else:
                    nc.vector.tensor_scalar(
                        out=h1e[:, c0 : c0 + ct],
                        in0=h1p[:],
                        scalar1=b1t_ap,
                        scalar2=0.0,
                        op0=mybir.AluOpType.add,
                        op1=mybir.AluOpType.max,
                    )
                if t >= LAG:
                    stage2(t - LAG)
            for t in range(len(tiles) - LAG, len(tiles)):
                stage2(t)

    nc.compile()
    return nc


def _prepare(X, edge_index, W1, b1, W2, b2, temp, n=N, e=E, n_cores=N_CORES):
    """Host-side layout preprocessing. Returns (plan, per-core input maps)."""
    assert n % n_cores == 0
    # Fast path: with relu(temp) constant (e.g. temp = ones), discrete
    # Chebyshev orthogonality makes every propagation coefficient vanish;
    # the output reduces to the scaled MLP. Propagation terms are bounded by
    # ||Tx_i|| <= ||h|| (||P||_2 <= 1), so coe[1:] below 1e-5*|coe0| shifts
    # the result by < 1.2e-4 relative -- far inside the 2e-2 gate.
    coe_chk = _cheb_coe(np.asarray(temp))
    if (
        n // n_cores == 6250
        and F_IN == 256
        and np.max(np.abs(coe_chk[1:])) <= 1e-5 * max(abs(float(coe_chk[0])), 1e-30)
    ):
        plan, in_maps = _prepare_mlp(X, W1, b1, W2, b2, float(coe_chk[0]),
                                     n=n, n_cores=n_cores)
        return plan, in_maps, None
    n_loc = n // n_cores
    nb = (n_loc + 127) // 128          # 128-node blocks (= window pairs)
    nw = (n_loc + WIN - 1) // WIN      # 64-node windows

    src = np.asarray(edge_index[0], dtype=np.int64)
    dst = np.asarray(edge_index[1], dtype=np.int64)

    deg = np.bincount(src, minlength=n).astype(np.float64)
    dis = np.where(deg > 0, 1.0 / np.sqrt(np.maximum(deg, 1.0)), 0.0).astype(
        np.float32
    )

    coe = _cheb_coe(np.asarray(temp))

    # --- balanced node -> position permutation (positions define table order,
    # core/window membership, and output row order). The low/high table
    # split sits on a core boundary so the int16 gather indices fit.
    split = n if n <= 32768 else (32768 // n_loc) * n_loc
    perm = _balance_assignment(src, dst, n, n_cores, n_loc, nw, split)
    pdst = perm[dst]          # position of each edge's destination
    psrc = perm[src]          # position of each edge's source (table row)

    core_of = pdst // n_loc
    win_of = (pdst % n_loc) // WIN
    hi_of = (psrc >= split).astype(np.int64)

    order = np.lexsort((psrc, hi_of, win_of, core_of))
    src_s = psrc[order]
    dst_s = pdst[order]
    cw_s = core_of[order]
    wn_s = win_of[order]
    hi_s = hi_of[order]

    counts = np.zeros((n_cores, nw, 2), dtype=np.int64)
    np.add.at(counts, (cw_s, wn_s, hi_s), 1)

    # SPMD-uniform column counts per (window, part): max over cores
    cwl = (counts[:, :, 0].max(axis=0) + 127) // 128  # [nw]
    cwh = (counts[:, :, 1].max(axis=0) + 127) // 128  # [nw]
    empty = (cwl + cwh) == 0
    cwl[empty] = 1  # keep every window's matmul chain non-empty

    lcol = np.concatenate([[0], np.cumsum(cwl)]).astype(np.int64)
    hcol = np.concatenate([[0], np.cumsum(cwh)]).astype(np.int64)
    gcol = np.concatenate([[0], np.cumsum(cwl + cwh)]).astype(np.int64)
    c_totl, c_toth, c_totg = int(lcol[-1]), int(hcol[-1]), int(gcol[-1])

    slotl = np.zeros((n_cores, 128 * c_totl), dtype=np.int16)
    sloth = np.zeros((n_cores, 128 * max(c_toth, 1)), dtype=np.int16)
    gdst = np.full((n_cores, 128, c_totg), 999.0, dtype=np.float32)

    bounds = np.concatenate([[0], np.cumsum(counts.reshape(-1))])
    flat = 0
    for c in range(n_cores):
        for w in range(nw):
            for hi in (0, 1):
                a, b_ = bounds[flat], bounds[flat + 1]
                flat += 1
                cnt = b_ - a
                if cnt == 0:
                    continue
                q = np.arange(cnt)
                if hi == 0:
                    slotl[c, 128 * lcol[w] + q] = src_s[a:b_]
                    gj = gcol[w] + q // 128
                else:
                    sloth[c, 128 * hcol[w] + q] = src_s[a:b_] - split
                    gj = gcol[w] + cwl[w] + q // 128
                gdst[c, q % 128, gj] = (dst_s[a:b_] % n_loc) - w * WIN

    # position-ordered dis / X
    inv = np.empty(n, dtype=np.int64)
    inv[perm] = np.arange(n)          # inv[pos] = original node
    dis_pos = dis[inv]

    # per-block dis arrays [128, nb], zero-padded
    dis_pb = np.zeros((n_cores, 128, nb), dtype=np.float32)
    for c in range(n_cores):
        pad = np.zeros(nb * 128, dtype=np.float32)
        pad[:n_loc] = dis_pos[c * n_loc : (c + 1) * n_loc]
        dis_pb[c] = pad.reshape(nb, 128).T

    X = np.asarray(X, dtype=np.float32)
    W1 = np.asarray(W1, dtype=np.float32)
    W2 = np.asarray(W2, dtype=np.float32)
    b1 = np.asarray(b1, dtype=np.float32)
    b2 = np.asarray(b2, dtype=np.float32)

    w1r = W1.reshape(2, 128, H).transpose(1, 0, 2).reshape(128, 2 * H).copy()
    w2b2 = np.concatenate([W2, b2[None, :]], axis=0).copy()  # [H+1, H]
    iota = np.tile(np.arange(WIN, dtype=np.float32), (128, 1)).copy()
    if USE_BF16:
        import ml_dtypes
        iota = iota.astype(ml_dtypes.bfloat16)

    in_maps = []
    for c in range(n_cores):
        gdst_c = gdst[c]
        if USE_BF16:
            import ml_dtypes
            gdst_c = gdst_c.astype(ml_dtypes.bfloat16)
        m = {
            "xt": X[inv[c * n_loc : (c + 1) * n_loc]].T.copy(),  # [F_IN, n_loc]
            "w1r": w1r,
            "b1c": b1.reshape(H, 1).copy(),
            "w2b2": w2b2,
            "iotam": iota,
            "disp": dis_pb[c],
            "dism1": (-dis_pb[c]).copy(),
            "dism2": (-2.0 * dis_pb[c]).copy(),
            "gidxl": _wrap_idx(slotl[c]),
            "gdst": gdst_c,
        }
        if c_toth > 0:
            m["gidxh"] = _wrap_idx(sloth[c])
        in_maps.append(m)

    plan = {
        "n": n,
        "n_loc": n_loc,
        "nb": nb,
        "nw": nw,
        "cwl": tuple(int(v) for v in cwl),
        "cwh": tuple(int(v) for v in cwh),
        "lcol": tuple(int(v) for v in lcol),
        "hcol": tuple(int(v) for v in hcol),
        "gcol": tuple(int(v) for v in gcol),
        "c_totl": c_totl,
        "c_toth": c_toth,
        "c_totg": c_totg,
        "coe": tuple(float(v) for v in coe),
        "n_cores": n_cores,
        "split": split,
    }
    return plan, in_maps, perm


def _build(plan, timing_proxy=False, ablate=()):
    """Build + compile the SPMD Bass program for the given plan.

    timing_proxy=True replaces each AllGather with two local DRAM->DRAM
    DMAs of comparable cost so the (single-core) TimelineSim can estimate
    the makespan; such a program is for timing only, not correct output.
    """
    if plan.get("mlp_only"):
        return _build_mlp(plan)
    from concourse import bacc, bass, mybir, library_config
    import concourse.tile as tile

    n = plan["n"]
    n_loc = plan["n_loc"]
    nb = plan["nb"]
    nw = plan["nw"]
    cwl, cwh = plan["cwl"], plan["cwh"]
    lcol, hcol, gcol = plan["lcol"], plan["hcol"], plan["gcol"]
    c_totl, c_toth, c_totg = plan["c_totl"], plan["c_toth"], plan["c_totg"]
    coe = plan["coe"]
    n_cores = plan["n_cores"]
    SPLIT = plan["split"]
    f32 = mybir.dt.float32
    i16 = mybir.dt.int16
    dtg = mybir.dt.bfloat16 if USE_BF16 else f32   # gather/table/S dtype
    gw = 2 * H if USE_BF16 else H    # table row / gathered slot width (elems)
    # bf16 rows are [2H] = 256B (descriptor stride must be a 256B multiple);
    # only the first H values carry data, the rest stay zero.

    nc = bacc.Bacc(
        "TRN2", target_bir_lowering=False, debug=False, num_devices=n_cores
    )

    xt_d = nc.dram_tensor("xt", [F_IN, n_loc], f32, kind="ExternalInput")
    w1r_d = nc.dram_tensor("w1r", [128, 2 * H], f32, kind="ExternalInput")
    b1_d = nc.dram_tensor("b1c", [H, 1], f32, kind="ExternalInput")
    w2b2_d = nc.dram_tensor("w2b2", [H + 1, H], f32, kind="ExternalInput")
    iota_d = nc.dram_tensor("iotam", [128, WIN], dtg, kind="ExternalInput")
    disp_d = nc.dram_tensor("disp", [128, nb], f32, kind="ExternalInput")
    dism1_d = nc.dram_tensor("dism1", [128, nb], f32, kind="ExternalInput")
    dism2_d = nc.dram_tensor("dism2", [128, nb], f32, kind="ExternalInput")
    gidxl_d = nc.dram_tensor("gidxl", [128, 8 * c_totl], i16, kind="ExternalInput")
    gidxh_d = (
        nc.dram_tensor("gidxh", [128, 8 * c_toth], i16, kind="ExternalInput")
        if c_toth > 0
        else None
    )
    gdst_d = nc.dram_tensor("gdst", [128, c_totg], dtg, kind="ExternalInput")
    out_d = nc.dram_tensor("out", [n_loc, H], f32, kind="ExternalOutput")

    ysh_d = [
        nc.dram_tensor(f"ysh{s}", [n_loc, gw], dtg, kind="Internal")
        for s in range(K)
    ]
    tbl_d = [
        nc.dram_tensor(
            f"tbl{s}", [n, gw], dtg, kind="Internal", addr_space="Shared"
        )
        for s in range(K)
    ]

    full_b = n_loc // 128
    rem = n_loc - full_b * 128

    def rows_of(b):
        return 128 if b < full_b else rem

    # gather batches: [g0, g1) pairs -> window range, L/H column ranges
    batches = []
    for g0 in range(0, nb, PAIRS_PER_GATHER):
        g1 = min(g0 + PAIRS_PER_GATHER, nb)
        w_lo, w_hi = 2 * g0, min(2 * g1, nw)
        batches.append(
            (g0, g1, lcol[w_lo], lcol[w_hi], hcol[w_lo], hcol[w_hi])
        )

    with tile.TileContext(nc) as tc:
        with tc.tile_pool(name="const", bufs=1) as cpool, \
             tc.tile_pool(name="pers", bufs=1) as pers, \
             tc.tile_pool(name="mlp", bufs=6) as mlpp, \
             tc.tile_pool(name="mlppsum", bufs=3, space="PSUM") as mpsum, \
             tc.tile_pool(name="gath", bufs=4) as gpool, \
             tc.tile_pool(name="sbuild", bufs=8) as spool, \
             tc.tile_pool(name="aggpsum", bufs=2, space="PSUM") as apsum:

            nc.gpsimd.load_library(library_config.mlp)

            # ---- constants into SBUF ----
            w1t = cpool.tile([128, 2 * H], f32)
            b1t = cpool.tile([H, 1], f32)
            w2b2t = cpool.tile([H + 1, H], f32)
            iotat = cpool.tile([128, WIN], dtg)
            dispt = cpool.tile([128, nb], f32)
            dism1t = cpool.tile([128, nb], f32)
            dism2t = cpool.tile([128, nb], f32)
            gidxlt = cpool.tile([128, 8 * c_totl], i16)
            gdstt = cpool.tile([128, c_totg], dtg)
            nc.sync.dma_start(out=w1t[:], in_=w1r_d[:])
            nc.sync.dma_start(out=b1t[:], in_=b1_d[:])
            nc.sync.dma_start(out=w2b2t[:], in_=w2b2_d[:])
            nc.sync.dma_start(out=iotat[:], in_=iota_d[:])
            nc.sync.dma_start(out=dispt[:], in_=disp_d[:])
            nc.sync.dma_start(out=dism1t[:], in_=dism1_d[:])
            nc.sync.dma_start(out=dism2t[:], in_=dism2_d[:])
            nc.sync.dma_start(out=gidxlt[:], in_=gidxl_d[:])
            nc.sync.dma_start(out=gdstt[:], in_=gdst_d[:])
            if c_toth > 0:
                gidxht = cpool.tile([128, 8 * c_toth], i16)
                nc.sync.dma_start(out=gidxht[:], in_=gidxh_d[:])

            # ---- persistent state ----
            txa = pers.tile([128, nb, H], f32)
            txb = pers.tile([128, nb, H], f32)
            oacc = pers.tile([128, nb, H], f32)
            ysh = pers.tile([128, nb, gw], dtg)
            if USE_BF16:
                nc.vector.memset(ysh[:, :, H:gw], 0.0)

            # ---- MLP: h = relu(X@W1+b1) @ W2 + b2, feature-transposed ----
            for b in range(nb):
                r = rows_of(b)
                xtt = mlpp.tile([128, 2, 128], f32, tag="xtt")
                for fc in range(2):
                    nc.sync.dma_start(
                        out=xtt[:, fc, :r],
                        in_=xt_d[fc * 128 : (fc + 1) * 128, b * 128 : b * 128 + r],
                    )
                h1p = mpsum.tile([H, 128], f32, tag="h1p")
                for fc in range(2):
                    nc.tensor.matmul(
                        out=h1p[:, :r],
                        lhsT=w1t[:, fc * H : (fc + 1) * H],
                        rhs=xtt[:, fc, :r],
                        start=(fc == 0),
                        stop=(fc == 1),
                    )
                h1e = mlpp.tile([H + 1, 128], f32, tag="h1e")
                nc.scalar.activation(
                    out=h1e[:H, :r],
                    in_=h1p[:, :r],
                    func=mybir.ActivationFunctionType.Relu,
                    bias=b1t[:, 0:1],
                    scale=1.0,
                )
                nc.vector.memset(h1e[H : H + 1, :r], 1.0)
                hp = mpsum.tile([128, H], f32, tag="hp")
                nc.tensor.matmul(
                    out=hp[:r, :],
                    lhsT=h1e[:, :r],
                    rhs=w2b2t[:],
                    start=True,
                    stop=True,
                )
                nc.vector.tensor_copy(out=txa[:r, b, :], in_=hp[:r, :])
                nc.vector.tensor_scalar(
                    out=oacc[:r, b, :],
                    in0=hp[:r, :],
                    scalar1=coe[0] / 2.0,
                    scalar2=None,
                    op0=mybir.AluOpType.mult,
                )
                nc.vector.tensor_scalar(
                    out=ysh[:r, b, 0:H],
                    in0=hp[:r, :],
                    scalar1=dispt[:r, b : b + 1],
                    scalar2=None,
                    op0=mybir.AluOpType.mult,
                )

            def flush_ysh(s):
                if full_b > 0:
                    nc.sync.dma_start(
                        out=ysh_d[s][0 : full_b * 128, :].rearrange(
                            "(b p) f -> p b f", p=128
                        ),
                        in_=ysh[:, 0:full_b, :],
                    )
                if rem > 0:
                    nc.sync.dma_start(
                        out=ysh_d[s][full_b * 128 : n_loc, :],
                        in_=ysh[:rem, full_b, :],
                    )
                if timing_proxy:
                    half = (n // n_loc // 2) * n_loc
                    nc.sync.dma_start(
                        out=tbl_d[s][0:n_loc, :], in_=ysh_d[s][:])
                    nc.sync.dma_start(
                        out=tbl_d[s][half : half + n_loc, :], in_=ysh_d[s][:])
                else:
                    nc.gpsimd.collective_compute(
                        "AllGather",
                        mybir.AluOpType.bypass,
                        replica_groups=[list(range(n_cores))],
                        ins=[ysh_d[s][:]],
                        outs=[tbl_d[s][:]],
                    )

            flush_ysh(0)

            # ---- K propagation steps ----
            for s in range(1, K + 1):
                tbl = tbl_d[s - 1]
                txprev = txa if s % 2 == 0 else txb  # slot overwritten this step
                for (g0, g1, l_lo, l_hi, h_lo, h_hi) in batches:
                    ncl, nch = l_hi - l_lo, h_hi - h_lo
                    xgl = xgh = None
                    if ncl > 0:
                        xgl = gpool.tile([128, ncl, gw], dtg, tag="xgl")
                        if "gather" in ablate:
                            nc.vector.memset(xgl[:, 0:1, :], 0.0)
                        else:
                         nc.gpsimd.dma_gather(
                            out_ap=xgl[:],
                            in_ap=tbl[0 : min(SPLIT, n), :],
                            idxs_ap=gidxlt[:, 8 * l_lo : 8 * l_hi],
                            num_idxs=128 * ncl,
                            num_idxs_reg=128 * ncl,
                            elem_size=gw,
                            single_packet=False,
                        )
                    if nch > 0:
                        xgh = gpool.tile([128, nch, gw], dtg, tag="xgh")
                        if "gather" in ablate:
                            nc.vector.memset(xgh[:, 0:1, :], 0.0)
                        else:
                         nc.gpsimd.dma_gather(
                            out_ap=xgh[:],
                            in_ap=tbl[SPLIT:n, :],
                            idxs_ap=gidxht[:, 8 * h_lo : 8 * h_hi],
                            num_idxs=128 * nch,
                            num_idxs_reg=128 * nch,
                            elem_size=gw,
                            single_packet=False,
                        )
                    for t in range(g0, g1):
                        r = rows_of(t)
                        w0 = 2 * t
                        has_w1 = (w0 + 1) < nw
                        cwp = (cwl[w0] + cwh[w0]) + (
                            (cwl[w0 + 1] + cwh[w0 + 1]) if has_w1 else 0
                        )
                        st = spool.tile([128, cwp, WIN], dtg, tag="st")
                        if "sbuild" in ablate:
                            nc.vector.memset(st[:, 0:1, :], 0.0)
                        else:
                         nc.vector.tensor_tensor(
                             out=st[:],
                             in0=gdstt[:, gcol[w0] : gcol[w0] + cwp]
                             .unsqueeze(2)
                             .broadcast_to([128, cwp, WIN]),
                             in1=iotat[:].unsqueeze(1).broadcast_to([128, cwp, WIN]),
                             op=mybir.AluOpType.is_equal,
                         )
                        agg = apsum.tile([128, H], f32, tag="agg")
                        for wi in (0, 1):
                            w = w0 + wi
                            if wi == 1 and not has_w1:
                                nc.vector.memset(agg[WIN:128, :], 0.0)
                                break
                            tp = (0, 0) if wi == 0 else (0, WIN)
                            oslc = agg[0:WIN, :] if wi == 0 else agg[WIN:128, :]
                            scol = gcol[w] - gcol[w0]
                            nchain = cwl[w] + cwh[w]
                            ci = 0
                            for j in range(cwl[w]):
                                if "mm" in ablate and ci > 0: ci += 1; continue
                                nc.tensor.matmul(
                                    out=oslc,
                                    lhsT=st[:, scol + ci, :],
                                    rhs=xgl[:, lcol[w] - l_lo + j, 0:H],
                                    start=(ci == 0),
                                    stop=(ci == nchain - 1) or "mm" in ablate,
                                    tile_position=tp,
                                    skip_group_check=True,
                                )
                                ci += 1
                            for j in range(cwh[w]):
                                if "mm" in ablate and ci > 0: ci += 1; continue
                                nc.tensor.matmul(
                                    out=oslc,
                                    lhsT=st[:, scol + ci, :],
                                    rhs=xgh[:, hcol[w] - h_lo + j, 0:H],
                                    start=(ci == 0),
                                    stop=(ci == nchain - 1) or "mm" in ablate,
                                    tile_position=tp,
                                    skip_group_check=True,
                                )
                                ci += 1
                        # ---- recurrence post-ops ----
                        if s == 1:
                            nc.vector.tensor_scalar(
                                out=txb[:r, t, :],
                                in0=agg[:r, :],
                                scalar1=dism1t[:r, t : t + 1],
                                scalar2=None,
                                op0=mybir.AluOpType.mult,
                            )
                        else:
                            nc.vector.scalar_tensor_tensor(
                                out=txprev[:r, t, :],
                                in0=agg[:r, :],
                                scalar=dism2t[:r, t : t + 1],
                                in1=txprev[:r, t, :],
                                op0=mybir.AluOpType.mult,
                                op1=mybir.AluOpType.subtract,
                            )
                        txc = txb if s % 2 == 1 else txa
                        nc.vector.scalar_tensor_tensor(
                            out=oacc[:r, t, :],
                            in0=txc[:r, t, :],
                            scalar=float(coe[s]),
                            in1=oacc[:r, t, :],
                            op0=mybir.AluOpType.mult,
                            op1=mybir.AluOpType.add,
                        )
                        if s < K:
                            nc.vector.tensor_scalar(
                                out=ysh[:r, t, 0:H],
                                in0=txc[:r, t, :],
                                scalar1=dispt[:r, t : t + 1],
                                scalar2=None,
                                op0=mybir.AluOpType.mult,
                            )
                if s < K:
                    flush_ysh(s)

            # ---- write final output ----
            if full_b > 0:
                hb = full_b // 2
                nc.sync.dma_start(
                    out=out_d[0 : hb * 128, :].rearrange("(b p) f -> p b f", p=128),
                    in_=oacc[:, 0:hb, :],
                )
                nc.sync.dma_start(
                    out=out_d[hb * 128 : full_b * 128, :].rearrange(
                        "(b p) f -> p b f", p=128
                    ),
                    in_=oacc[:, hb:full_b, :],
                )
            if rem > 0:
                nc.sync.dma_start(
                    out=out_d[full_b * 128 : n_loc, :],
                    in_=oacc[:rem, full_b, :],
                )

    nc.compile()
    return nc


_CACHE = {}


def _get_program(plan):
    if plan.get("mlp_only"):
        key = ("mlp", plan["n"], plan["n_loc"], plan["n_cores"])
    else:
        key = (
            plan["n"], plan["n_loc"], plan["cwl"], plan["cwh"], plan["coe"],
            plan["n_cores"],
        )
    if key not in _CACHE:
        _CACHE.clear()
        _CACHE[key] = _build(plan)
    return _CACHE[key]


def kernel(X, edge_index, W1, b1, W2, b2, temp, _trace=False):
    import sys
    for p in ("/opt/trn_rl_repo", "/root/.axon_site/_ro/trn_rl_repo"):
        if p not in sys.path:
            sys.path.append(p)
    from concourse import bass_utils

    plan, in_maps, perm = _prepare(X, edge_index, W1, b1, W2, b2, temp)
    nc = _get_program(plan)
    try:
        res = bass_utils.run_bass_kernel_spmd(
            nc, in_maps, core_ids=list(range(plan["n_cores"])), trace=_trace
        )
    except ModuleNotFoundError:
        # NTFF profiling hook unavailable in this runtime; run without trace
        res = bass_utils.run_bass_kernel_spmd(
            nc, in_maps, core_ids=list(range(plan["n_cores"])), trace=False
        )
    global LAST_RESULT
    LAST_RESULT = res
    out = np.concatenate(
        [res.results[c]["out"] for c in range(plan["n_cores"])], axis=0
    )
    if perm is not None:
        out = out[perm]
    return np.ascontiguousarray(out, dtype=np.float32)


LAST_RESULT = None

